# revision 49
# baseline (speedup 1.0000x reference)
"""Multi-head causal attention (B=4, S=2048, D=512, H=8) on 8 trn2 NeuronCores.

Sharding: core c -> batch b = c//2, head group hg = c%2 (heads 4*hg .. 4*hg+3).
Each core computes its 4 heads' attention and a partial output projection
(sum over its heads of out_h @ Wo[h-rows]); host sums the two partials per
batch.  The hg==1 core receives a zero bo so the bias is added exactly once.

Device layout (per core; matmul operands bf16, PSUM fp32):
  xT [512, 2048] = x[b].T.  Q^T/K^T per head-pair p are [e2=128, S] with the
  pair's two heads stacked on partition halves; scores are computed
  TRANSPOSED, sT[k, q] = K q^T, with the two heads' K=64-contraction matmuls
  row-tiled into the top/bottom halves of the PE array (concurrent), both
  written into one 2-bank PSUM tile so a single exp (ScalarE, 3D AP) covers
  them.  The causal diagonal 128x128 block is masked by multiplying exp with
  a 0/1 upper-triangular matrix (GpSimd hi0 / DVE hi1).
  V' = [V | 1 | pad] is padded to 128 columns (FWL-eligible LDWEIGHTS);
  attn @ V' also yields the softmax normalizer Z in psum row 64.
  Normalization: both heads' Z rows copied into one [1,1024] tile, one
  reciprocal_approx_fast, broadcast across 64 partitions by a DRAM
  round-trip DMA (zero-stride read), then two tensor_tensor multiplies.
  The output projection is TRANSPOSED (stationary = Wo d-chunk, output
  prT[d-chunk, q] with d on partitions) so the bias is per-partition: a
  tensor_scalar (DVE) mid-kernel, ScalarE activation-add in the endgame
  where DVE is the critical chain.  Output dram layout is [D, S]; the host
  transposes while summing the two partials.

Scheduling: the steady-state rate limiter is ScalarE (exp).  All QKV
projection units AND output-projection chunks are fill units drained into
the attention j-loops' PE slack in strict deadline order (QK for the next
block first, V chunks by their first diagonal-attnV use, proj chunks
opportunistically), with per-block barriers so a proj unit is never
emitted before its block's normalize.  This removes the inter-block PE
bursts that starved the exp stream.

Startup: the critical prefix (xT[:, 0:512] x4 d-chunks, wq/wk pair 0,
bq/bk) is split across all four DMA queues; a dummy Exp activation at t=0
pulls the ACT table load off the critical path; ~34 N=128 dummy matmuls
keep the PE busy from t~0.2us so the HAM clock gate flips to 2.4 GHz at
~3.5us (instead of ~24us) and every real matmul runs warm.  Bulk xT loads
are s4-major so qT/kT for later blocks are computable as early as
possible.  Endgame: normalize chains split across ScalarE (free after the
last exp) and DVE, dummy matmuls keep the clock warm through the DVE
chain, and the final four output DMAs go out on four different queues.
"""

import numpy as np
import ml_dtypes

B, S, D, H = 4, 2048, 512, 8
E = D // H  # 64
NCORES = 8
SCALE = float(D) ** -0.5
BF16 = ml_dtypes.bfloat16

_CACHE: dict = {}


def _build_bass():
    import concourse.bass as bass
    import concourse.mybir as mybir
    import concourse.tile as tile
    from concourse import bacc
    from contextlib import ExitStack

    fp32 = mybir.dt.float32
    bf16 = mybir.dt.bfloat16
    Act = mybir.ActivationFunctionType
    Alu = mybir.AluOpType

    nc = bacc.Bacc(
        "TRN2",
        target_bir_lowering=False,
        debug=False,
        num_devices=NCORES,
    )

    xT = nc.dram_tensor("xT", [D, S], bf16, kind="ExternalInput").ap()
    wq = nc.dram_tensor("wq", [2, 4, 128, 128], bf16, kind="ExternalInput").ap()
    wk = nc.dram_tensor("wk", [2, 4, 128, 128], bf16, kind="ExternalInput").ap()
    bq = nc.dram_tensor("bq", [2, 128, 1], fp32, kind="ExternalInput").ap()
    bk = nc.dram_tensor("bk", [2, 128, 1], fp32, kind="ExternalInput").ap()
    wv = nc.dram_tensor("wv", [4, 128, 256], bf16, kind="ExternalInput").ap()
    wo = nc.dram_tensor("wo", [2, 128, 512], bf16, kind="ExternalInput").ap()
    boc = nc.dram_tensor("boc", [128, 4], fp32, kind="ExternalInput").ap()
    um = nc.dram_tensor("um", [128, 128], bf16, kind="ExternalInput").ap()
    out = nc.dram_tensor("out", [D, S], bf16, kind="ExternalOutput").ap()

    def bcast_ap(src: bass.AP, parts: int, n: int) -> bass.AP:
        """DRAM [1, n] row replicated to [parts, n] via a zero-stride dim."""
        return bass.AP(src.tensor, src.offset, [[0, parts], [1, n]])

    with tile.TileContext(nc) as tc, ExitStack() as ctx:
        const = ctx.enter_context(tc.tile_pool(name="const", bufs=1))
        big = ctx.enter_context(tc.tile_pool(name="big", bufs=1))
        work = ctx.enter_context(tc.tile_pool(name="work", bufs=3))
        psum = ctx.enter_context(tc.tile_pool(name="psum", bufs=2, space="PSUM"))
        dram = ctx.enter_context(tc.tile_pool(name="dram", bufs=2, space="DRAM"))

        # ---- SBUF residents ----
        xT_sb = const.tile([128, 4 * S], bf16)  # d-chunk major
        wq_sb = const.tile([128, 8 * 128], bf16)  # (p, dc) major
        wk_sb = const.tile([128, 8 * 128], bf16)
        wv_sb = const.tile([128, 4 * 256], bf16)  # dc major
        wo_sb = const.tile([128, 2 * 512], bf16)  # pair major
        bq_sb = const.tile([128, 2], fp32)
        bk_sb = const.tile([128, 2], fp32)
        um_sb = const.tile([128, 128], bf16)
        bo_col = const.tile([128, 4], fp32)
        warm = const.tile([128, 128], bf16)  # dummy matmul operand (zeros)
        dum_out = const.tile([1, 32], fp32)
        onef = const.tile([1, 64], fp32)  # fp32 ones row for tail PE-broadcast

        # persistent intermediates
        qT_sb = big.tile([128, 2 * S], bf16)  # pair-major; head halves on partitions
        kT_sb = big.tile([128, 2 * S], bf16)
        Vp_sb = big.tile([128, 4 * 16 * 128], bf16)  # (head, k-chunk) major
        Vp4 = Vp_sb.rearrange("d (h j e) -> d h j e", h=4, j=16)
        outT0 = big.tile([128, S], bf16)
        outT1 = big.tile([128, S], bf16)
        outT = [outT0, outT1]

        def load_xt(dc, lo, hi, eng):
            eng.dma_start(
                xT_sb[:, dc * S + lo : dc * S + hi],
                xT[dc * 128 : (dc + 1) * 128, lo:hi],
            )

        def load_w(dst, src, p, eng):
            eng.dma_start(
                dst[:, p * 512 : p * 512 + 512].rearrange("d (a e) -> d a e", a=4),
                src[p].rearrange("c d e -> d c e"),
            )

        # ---- t=0 per-engine programs ----
        # vector: warm operand, V' ones (st0..3), onef
        nc.vector.memset(warm, 0.0)
        nc.vector.memset(onef, 1.0)
        # scalar: xt2+xt3 prefix issues, V' pad zero (st0..3), dummy exp ->
        # ACT table loads at ~2us instead of right before the first real exp
        load_xt(2, 0, 512, nc.scalar)
        load_xt(3, 0, 512, nc.scalar)
        nc.scalar.memzero(Vp4[:, :, 0:4, 64:128])
        nc.vector.memset(Vp4[:, :, 0:4, 64:65], 1.0)
        nc.scalar.activation(dum_out, warm[0:1, 0:32], Act.Exp)
        # sync: xt0, wq p0, bq, bk, um, wq/wk p1, then s4-major bulk (dc 0,1)
        load_xt(0, 0, 512, nc.sync)
        load_w(wq_sb, wq, 0, nc.sync)
        nc.sync.dma_start(bq_sb, bq.rearrange("p d e -> d (p e)"))
        nc.sync.dma_start(bk_sb, bk.rearrange("p d e -> d (p e)"))
        nc.sync.dma_start(um_sb, um)
        load_w(wq_sb, wq, 1, nc.sync)
        load_w(wk_sb, wk, 1, nc.sync)
        for s4 in (1, 2, 3):
            for dc in (0, 1):
                load_xt(dc, s4 * 512, s4 * 512 + 512, nc.sync)
        # gpsimd: xt1, wk p0, wv, bv, bulk s4=1 (dc 2,3), wo, boc,
        # V' pad zero + ones (st4..15), bulk s4=2,3 (dc 2,3)
        load_xt(1, 0, 512, nc.gpsimd)
        load_w(wk_sb, wk, 0, nc.gpsimd)
        nc.gpsimd.dma_start(
            wv_sb.rearrange("d (a e) -> d a e", a=4),
            wv.rearrange("c d e -> d c e"),
        )
        for dc in (2, 3):
            load_xt(dc, 512, 1024, nc.gpsimd)
        nc.gpsimd.dma_start(
            wo_sb.rearrange("d (a e) -> d a e", a=2),
            wo.rearrange("p d e -> d p e"),
        )
        nc.gpsimd.dma_start(bo_col, boc)
        nc.gpsimd.memset(Vp4[:, :, 4:16, 64:128], 0.0)
        nc.gpsimd.memset(Vp4[:, :, 4:16, 64:65], 1.0)
        for s4 in (2, 3):
            for dc in (2, 3):
                load_xt(dc, s4 * 512, s4 * 512 + 512, nc.gpsimd)

        # ---- PE warmup: keep the array busy from t~0.2us so the HAM clock
        # gate flips to 8/8 at ~3.5us and every real matmul runs at 2.4 GHz.
        wps = psum.tile([128, 512], fp32, tag="o0", name="wps")
        for _ in range(32):
            nc.tensor.matmul(
                wps[:, 0:128], lhsT=warm, rhs=warm, start=True, stop=True
            )

        # ---- QKV ----
        def emit_qk(p, which, s4):
            w_sb, b_sb, scl, dst, tag = (
                (wq_sb, bq_sb, SCALE, qT_sb, "o0")
                if which == "q"
                else (wk_sb, bk_sb, 1.0, kT_sb, "o1")
            )
            mm_ps = psum.tile([128, 512], fp32, tag=tag, name="mm_ps")
            for dc in range(4):
                i = p * 4 + dc
                nc.tensor.matmul(
                    mm_ps,
                    lhsT=w_sb[:, i * 128 : (i + 1) * 128],
                    rhs=xT_sb[:, dc * S + s4 * 512 : dc * S + s4 * 512 + 512],
                    start=(dc == 0),
                    stop=(dc == 3),
                )
            nc.vector.tensor_scalar(
                out=dst[:, p * S + s4 * 512 : p * S + s4 * 512 + 512],
                in0=mm_ps,
                scalar1=scl,
                scalar2=b_sb[:, p : p + 1],
                op0=Alu.mult,
                op1=Alu.add,
            )

        def emit_v(st):
            v_ps = psum.tile(
                [128, 256], fp32, tag=("o0" if st % 2 == 0 else "o1"), name="v_ps"
            )
            for dc in range(4):
                nc.tensor.matmul(
                    v_ps,
                    lhsT=xT_sb[:, dc * S + st * 128 : dc * S + st * 128 + 128],
                    rhs=wv_sb[:, dc * 256 : (dc + 1) * 256],
                    start=(dc == 0),
                    stop=(dc == 3),
                )
            v4 = v_ps.rearrange("d (h e) -> d h e", h=4)
            # bv is folded into the output-projection bias on the host
            # (out = attnV(v)/Z + bv per head => bv @ Wo adds to bo)
            nc.vector.tensor_copy(Vp4[:, :, st, 0:64], v4)

        # ---- transposed output projection: prT[d-chunk, q-block] ----
        def emit_proj_chunk(m4, dc, eng, tail=False):
            pr = psum.tile(
                [128, 512], fp32, tag=("o0" if dc % 2 == 0 else "o1"), name="pr"
            )
            for p in range(2):
                nc.tensor.matmul(
                    pr,
                    lhsT=wo_sb[:, p * 512 + dc * 128 : p * 512 + dc * 128 + 128],
                    rhs=outT[p][:, m4 * 512 : (m4 + 1) * 512],
                    start=(p == 0),
                    stop=(p == 1),
                )
            pr_sb = work.tile([128, 512], bf16, tag="pr_sb", name="pr_sb")
            if tail:
                # ScalarE is free after the last exp; DVE is the tail chain
                nc.scalar.add(pr_sb, pr, bo_col[:, dc : dc + 1])
            else:
                nc.vector.tensor_scalar(
                    out=pr_sb,
                    in0=pr,
                    scalar1=1.0,
                    scalar2=bo_col[:, dc : dc + 1],
                    op0=Alu.mult,
                    op1=Alu.add,
                )
            eng.dma_start(out[dc * 128 : (dc + 1) * 128, m4 * 512 : (m4 + 1) * 512], pr_sb)

        # ---- fill-unit schedule (baseline-style: QKV units drained into
        # the attention j-loops, remainder flushed in a burst before each
        # block; the bursts keep PE density high enough that the HAM clock
        # gate never re-throttles mid-kernel) ----
        from collections import deque
        from functools import partial

        units = deque()
        attnv_need = {}
        need = {(0, 0): 0}
        for st in range(0, 4):
            units.append(partial(emit_v, st))
        attnv_need[(0, 0)] = 2
        attnv_need[(0, 2)] = 4
        units.append(partial(emit_qk, 1, "q", 0))
        units.append(partial(emit_qk, 1, "k", 0))
        need[(0, 1)] = len(units)
        # QK(0,*,1) ahead of V4..7: its matmuls + DVE copy then land ~2
        # exps before the (0,1)->(1,0) boundary, so the next block's first
        # scores are never gated on a just-emitted unit
        units.append(partial(emit_qk, 0, "q", 1))
        units.append(partial(emit_qk, 0, "k", 1))
        need[(1, 0)] = len(units)
        for st in range(4, 8):
            units.append(partial(emit_v, st))
        units.append(partial(emit_qk, 1, "q", 1))
        units.append(partial(emit_qk, 1, "k", 1))
        need[(1, 1)] = len(units)
        for m4n in (2, 3):
            for st in range(4 * m4n, 4 * m4n + 4):
                units.append(partial(emit_v, st))
            units.append(partial(emit_qk, 0, "q", m4n))
            units.append(partial(emit_qk, 0, "k", m4n))
            need[(m4n, 0)] = len(units)
            units.append(partial(emit_qk, 1, "q", m4n))
            units.append(partial(emit_qk, 1, "k", m4n))
            need[(m4n, 1)] = len(units)
        total_units = len(units)

        def pop_fill(k):
            while k > 0 and units:
                units.popleft()()
                k -= 1

        def flush_to(n):
            while total_units - len(units) < n:
                units.popleft()()

        # ---- attention (transposed scores) ----
        def emit_attn(m4, p, tail=False, inject=None):
            nj = 4 * (m4 + 1)
            inject = dict(inject or {})
            o_ps = [
                psum.tile([128, 512], fp32, tag=f"o{hi}", name=f"o_ps{hi}")
                for hi in range(2)
            ]

            def normalize():
                # both heads' Z rows into one tile; single recip; DRAM
                # round-trip broadcast (zero-stride read); two muls.
                zrow2 = work.tile([1, 1024], fp32, tag="zrow", name="zrow")
                for hi in range(2):
                    nc.vector.tensor_copy(
                        zrow2[:, 512 * hi : 512 * hi + 512], o_ps[hi][64:65, :]
                    )
                rc = work.tile([1, 1024], fp32, tag="rc", name="rc")
                nc.vector.reciprocal_approx_fast(rc, zrow2)
                rcd = dram.tile([1, 1024], fp32, tag="rcd", name="rcd")
                nc.sync.dma_start(rcd, rc)
                bc = work.tile([64, 1024], fp32, tag="bc", name="bc")
                nc.gpsimd.dma_start(
                    bc, bass.AP(rcd.tensor, rcd.offset, [[0, 64], [1, 1024]])
                )
                for hi in range(2):
                    nc.vector.tensor_mul(
                        outT[p][64 * hi : 64 * hi + 64, m4 * 512 : m4 * 512 + 512],
                        o_ps[hi][0:64, :],
                        bc[:, 512 * hi : 512 * hi + 512],
                    )

            for jj in range(0, nj, 2):
                jpair = (jj, jj + 1)
                scps = {}
                exs = {}
                for j in jpair:
                    c0 = 128 * (j - 4 * m4) if j >= 4 * m4 else 0
                    scps[j] = psum.tile([128, 1024], fp32, tag="scp", name="scp")
                    for hi in range(2):
                        hp = 64 * hi
                        nc.tensor.matmul(
                            scps[j][:, 512 * hi + c0 : 512 * hi + 512],
                            lhsT=kT_sb[
                                hp : hp + 64,
                                p * S + j * 128 : p * S + j * 128 + 128,
                            ],
                            rhs=qT_sb[
                                hp : hp + 64,
                                p * S + m4 * 512 + c0 : p * S + m4 * 512 + 512,
                            ],
                            start=True,
                            stop=True,
                        )
                if jj >= 4 * m4 and (m4, jj) in attnv_need:
                    flush_to(attnv_need[(m4, jj)])
                for j in jpair:
                    c0 = 128 * (j - 4 * m4) if j >= 4 * m4 else 0
                    ex = work.tile([128, 1024], bf16, tag="ex", name="ex", bufs=6)
                    exs[j] = ex
                    nc.scalar.activation(
                        ex.rearrange("d (h q) -> d h q", h=2)[:, :, c0:512],
                        scps[j].rearrange("d (h q) -> d h q", h=2)[:, :, c0:512],
                        Act.Exp,
                    )
                    # fill rate matched to per-block PE slack: (1,*) blocks
                    # are attention+fill balanced, so only 1 unit per pair
                    if m4 == 1:
                        if j == jpair[1]:
                            pop_fill(1)
                    else:
                        pop_fill(2 if m4 == 0 else 1)
                    for hi in range(2):
                        if j >= 4 * m4:
                            eng = nc.gpsimd if hi == 0 else nc.vector
                            eng.tensor_mul(
                                ex[:, 512 * hi + c0 : 512 * hi + c0 + 128],
                                ex[:, 512 * hi + c0 : 512 * hi + c0 + 128],
                                um_sb,
                            )
                for hi in range(2):
                    for j in jpair:
                        c0 = 128 * (j - 4 * m4) if j >= 4 * m4 else 0
                        nc.tensor.matmul(
                            o_ps[hi][:, c0:512],
                            lhsT=Vp4[:, 2 * p + hi, j, 0:128],
                            rhs=exs[j][:, 512 * hi + c0 : 512 * hi + 512],
                            start=(j == 0),
                            stop=(j == nj - 1),
                            skip_group_check=True,
                        )
                for f in inject.pop(jj, ()):
                    f()
            if tail:
                # endgame: column halves pipelined against the final
                # diagonal pair; zrow copies on ScalarE (free now), recips
                # on DVE, 1/Z broadcast via fp32 ones-matmul on the PE
                # (bc_ps reuses the freed scp banks), dummy matmuls keep
                # the HAM clock warm through the DVE chain.
                # dummy matmuls run during the zrow/recip chain and keep the
                # clock gate at 8/8; their tile is allocated first so the
                # second bc_ps shares its buffer (PE-ordered, no stall)
                wps2 = psum.tile([128, 512], fp32, tag="scp", name="wps2")
                for _ in range(10):
                    nc.tensor.matmul(
                        wps2[:, 0:128], lhsT=warm, rhs=warm, start=True, stop=True
                    )
                bc_ps = [
                    psum.tile([64, 512], fp32, tag="scp", name="bc_ps")
                    for hi in range(2)
                ]
                halves = [(0, hi) for hi in range(2)] + [(256, hi) for hi in range(2)]
                rcs = {}
                for lo, hi in halves:
                    zrow = work.tile([1, 512], fp32, tag="zrt", name="zrt", bufs=4)
                    nc.scalar.copy(zrow[:, 0:256], o_ps[hi][64:65, lo : lo + 256])
                    rc = work.tile([1, 512], fp32, tag="rct", name="rct", bufs=4)
                    nc.vector.reciprocal_approx_fast(rc[:, 0:256], zrow[:, 0:256])
                    rcs[(lo, hi)] = rc
                for lo, hi in halves:
                    nc.tensor.matmul(
                        bc_ps[hi][:, lo : lo + 256],
                        lhsT=onef,
                        rhs=rcs[(lo, hi)][:, 0:256],
                        start=True,
                        stop=True,
                    )
                # keep the clock gate warm through the DVE chain so the
                # projection matmuls run at 2.4 GHz
                for _ in range(14):
                    nc.tensor.matmul(
                        wps2[:, 0:128], lhsT=warm, rhs=warm, start=True, stop=True
                    )
                bct = [
                    work.tile([64, 512], fp32, tag="bc", name="bct")
                    for hi in range(2)
                ]
                for lo, hi in halves:
                    if hi == 1:
                        nc.scalar.copy(
                            bct[hi][:, lo : lo + 256], bc_ps[hi][:, lo : lo + 256]
                        )
                    else:
                        nc.vector.tensor_copy(
                            bct[hi][:, lo : lo + 256], bc_ps[hi][:, lo : lo + 256]
                        )
                    nc.vector.tensor_mul(
                        outT[p][
                            64 * hi : 64 * hi + 64,
                            m4 * 512 + lo : m4 * 512 + lo + 256,
                        ],
                        o_ps[hi][0:64, lo : lo + 256],
                        bct[hi][:, lo : lo + 256],
                    )
            else:
                normalize()

        # ---- main loop ----
        emit_qk(0, "q", 0)
        emit_qk(0, "k", 0)
        proj_eng = [nc.sync, nc.gpsimd, nc.sync, nc.gpsimd]
        pending_proj = None
        for m4 in range(4):
            for p in range(2):
                flush_to(need[(m4, p)])
                # proj of the pending block: chunks 0,1 as a burst after the
                # p=0 block, chunks 2,3 injected after the p=1 block's first
                # pair (spreads the boundary burst; keeps the tail clear)
                inj = {}
                if p == 1 and pending_proj is not None:
                    inj[0] = [
                        partial(emit_proj_chunk, pending_proj, dc, proj_eng[dc])
                        for dc in (2, 3)
                    ]
                emit_attn(m4, p, tail=(m4 == 3 and p == 1), inject=inj)
                if p == 0 and pending_proj is not None:
                    for dc in (0, 1):
                        emit_proj_chunk(pending_proj, dc, proj_eng[dc])
                if p == 1:
                    pending_proj = m4
        while units:
            units.popleft()()
        # final projection: bias on ScalarE (kept clear of DMA issues),
        # chunks out alternating the sync/gpsimd queues
        tail_eng = [nc.sync, nc.gpsimd, nc.sync, nc.gpsimd]
        for dc in range(4):
            emit_proj_chunk(3, dc, tail_eng[dc], tail=True)

    nc.compile()
    return nc


def _get_bass():
    if "nc" not in _CACHE:
        _CACHE["nc"] = _build_bass()
    return _CACHE["nc"]


def make_in_maps(x, Wq, bq, Wk, bk, Wv, bv, Wo, bo):
    """Pack full fp32 inputs into 8 per-core input dicts.

    bv is folded into the per-core output-projection bias (boc)."""
    x = np.asarray(x, np.float32)
    Wq = np.asarray(Wq, np.float32)
    bq = np.asarray(bq, np.float32)
    Wk = np.asarray(Wk, np.float32)
    bk = np.asarray(bk, np.float32)
    Wv = np.asarray(Wv, np.float32)
    bv = np.asarray(bv, np.float32)
    Wo = np.asarray(Wo, np.float32)
    bo = np.asarray(bo, np.float32)

    um = np.triu(np.ones((128, 128), np.float32)).astype(BF16)  # keep q >= k

    in_maps = []
    for c in range(NCORES):
        b = c // 2
        hg = c % 2
        heads = [4 * hg + i for i in range(4)]

        xT_b = np.ascontiguousarray(x[b].T).astype(BF16)  # [512, 2048]

        wq_c = np.empty((2, 4, 128, 128), BF16)
        wk_c = np.empty((2, 4, 128, 128), BF16)
        bq_c = np.empty((2, 128, 1), np.float32)
        bk_c = np.empty((2, 128, 1), np.float32)
        wo_c = np.empty((2, 128, 512), BF16)
        for p in range(2):
            hA, hB = heads[2 * p], heads[2 * p + 1]
            blk_q = np.concatenate([Wq[hA], Wq[hB]], axis=1)  # [512, 128]
            blk_k = np.concatenate([Wk[hA], Wk[hB]], axis=1)
            for dc in range(4):
                wq_c[p, dc] = blk_q[dc * 128 : (dc + 1) * 128].astype(BF16)
                wk_c[p, dc] = blk_k[dc * 128 : (dc + 1) * 128].astype(BF16)
            bq_c[p, :, 0] = np.concatenate([bq[hA], bq[hB]]) * SCALE
            bk_c[p, :, 0] = np.concatenate([bk[hA], bk[hB]])
            wo_c[p] = np.concatenate(
                [Wo[E * hA : E * hA + E], Wo[E * hB : E * hB + E]], axis=0
            ).astype(BF16)

        wv_blk = np.concatenate([Wv[h] for h in heads], axis=1)  # [512, 256]
        wv_c = np.empty((4, 128, 256), BF16)
        for dc in range(4):
            wv_c[dc] = wv_blk[dc * 128 : (dc + 1) * 128].astype(BF16)

        # bv folds into the output bias: out_h = attnV(v)/Z + bv, so the
        # projection gains concat(bv[heads]) @ Wo[head rows]; bo itself is
        # added only by the hg==0 core.
        bo_core = sum(bv[h] @ Wo[E * h : E * h + E] for h in heads)
        if hg == 0:
            bo_core = bo_core + bo
        boc_c = np.ascontiguousarray(
            bo_core.astype(np.float32).reshape(4, 128).T
        )

        in_maps.append(
            {
                "xT": xT_b,
                "wq": wq_c,
                "wk": wk_c,
                "bq": bq_c,
                "bk": bk_c,
                "wv": wv_c,
                "wo": wo_c,
                "boc": boc_c,
                "um": um,
            }
        )
    return in_maps


def combine_outputs(parts):
    """parts: list of 8 [D, S] bf16 partials -> [B, S, D] fp32."""
    out = np.empty((B, S, D), np.float32)
    for b in range(B):
        acc = np.asarray(parts[2 * b], dtype=np.float32)
        acc += np.asarray(parts[2 * b + 1], dtype=np.float32)
        out[b] = acc.T
    return out


def kernel(**inputs):
    from concourse.bass_utils import run_bass_kernel_spmd

    nc = _get_bass()
    in_maps = make_in_maps(**inputs)
    res = run_bass_kernel_spmd(nc, in_maps, core_ids=list(range(NCORES)))
    return combine_outputs([r["out"] for r in res.results])
